# revision 15
# baseline (speedup 1.0000x reference)
"""GNN message-passing kernel for Trainium2 (8 NeuronCores, SPMD).

Computes: out = segment_sum(x[edge_index[0]], edge_index[1], num_segments=N)
  i.e. for each edge e: out[dst[e]] += x[src[e]]

Strategy (v2):
  - Shard destination nodes across 8 cores (R=12800 dst nodes/core).
  - One-time on device (outside the repeat loop): cast x fp32 -> x2 fp16
    stored [N, 128] (256B row stride, payload in cols 0..63).
  - Per edge: gather its 128B fp16 source row from x2 via the Q7 dma_gather
    (raw InstDMAGatherAnt: elem_size=64 fp16, elem_step=128 -> 128B
    descriptors; the non-transpose ucode path has no 256B granularity
    requirement).
  - Edges sorted by (bigwin-block, src chunk, bigwin, subwindow, dst).
    Chunks are 25000 src rows (int16 gather index limit). ONE gather call
    per (block of 5 bigwins, chunk) = 20 calls/core, so the ~1us fixed
    SWDGE generation cost per call amortizes. Padding slots use a VALID
    index 0 with dstl=-1 (one-hot row all zero -> contributes nothing),
    letting calls span group boundaries; only each call's trailing pad
    run uses -1 (trimmed for free by the ucode).
  - Scatter: per 128-edge tile, DVE tensor_scalar is_equal(iota_fp16,
    dstl_scalar) builds a [128,128] fp16 one-hot (TensorScalarPtr hits the
    4x_2p DVE mode); PE fp16 matmul accumulates msgs^T @ onehot into a
    feature-major PSUM bank [64 feat, 512 dst] per bigwin (5 banks live
    per block).
  - Flush PSUM -> SBUF outT [64, 12800] fp32 on the Activation engine ->
    single DMA out; host transposes.

The Bass program is identical across cores (SPMD): tile counts use the max
over cores; per-core variation lives in the idx / dstl input streams.
"""
import numpy as np

N_NODES = 100000
D_FEAT = 64
N_CORES = 8
R = 12800            # dst nodes per core (8*R = 102400 >= N_NODES)
BW = 512             # bigwin: dst nodes per PSUM bank
SW = 128             # subwindow: one-hot window width
CHUNK = 25000        # src rows per gather base (int16 idx limit 32767)
N_CHUNKS = 4
N_BW = R // BW       # 25 bigwins per core
N_SW = BW // SW      # 4 subwindows per bigwin
P = 128              # edge slots per tile
BLK = 5              # bigwins per block (PSUM banks live at once)
N_BLK = N_BW // BLK  # 5 blocks
GROUPS_PER_CALL = BLK * N_SW          # 20 (b, sw) groups per (blk, c) call
N_CALLS = N_BLK * N_CHUNKS            # 20 calls
N_GROUPS = N_BW * N_CHUNKS * N_SW     # 400


def cdiv(a, b):
    return -(-a // b)


def _preprocess(edge_index):
    """Sort/partition edges; build per-core device input streams and the
    (core-uniform) schedule. Purely graph-structural (no x values)."""
    src = np.asarray(edge_index[0], dtype=np.int64)
    dst = np.asarray(edge_index[1], dtype=np.int64)
    E = src.shape[0]

    core = dst // R
    dloc = dst % R
    b = dloc // BW                    # bigwin within core
    sw = (dloc % BW) // SW            # subwindow within bigwin
    ck = src // CHUNK                 # src chunk
    blk = b // BLK
    bl = b % BLK                      # bigwin within block

    # group key ordered call-major: call = (blk, ck), inside call (bl, sw)
    call = blk * N_CHUNKS + ck
    gkey = (call * BLK + bl) * N_SW + sw
    # per (core, gkey) counts
    cg = core * np.int64(N_GROUPS) + gkey
    counts = np.bincount(cg, minlength=N_CORES * N_GROUPS).reshape(
        N_CORES, N_GROUPS
    )
    max_counts = counts.max(axis=0)                     # [N_GROUPS]
    T_g = cdiv(max_counts, P)                           # tiles per group
    grp_tile_off = np.zeros(N_GROUPS, np.int64)
    np.cumsum(T_g[:-1], out=grp_tile_off[1:])
    grp_slot_off = grp_tile_off * P
    tot_tiles = int(T_g.sum())
    tot_slots = tot_tiles * P

    # call boundaries (slots): call k covers gkeys [k*GPC, (k+1)*GPC)
    call_g0 = np.arange(N_CALLS) * GROUPS_PER_CALL
    call_slot0 = grp_slot_off[call_g0]
    call_slots = np.array(
        [
            int(
                T_g[k * GROUPS_PER_CALL:(k + 1) * GROUPS_PER_CALL].sum() * P
            )
            for k in range(N_CALLS)
        ],
        np.int64,
    )

    # per-edge slot assignment: rank within each (core, gkey) group
    order_cg = np.argsort(cg, kind="stable")
    cg_s = cg[order_cg]
    grp_start = np.zeros(N_CORES * N_GROUPS + 1, np.int64)
    np.cumsum(counts.reshape(-1), out=grp_start[1:])
    rank = np.arange(E, dtype=np.int64) - grp_start[cg_s]
    slot = grp_slot_off[gkey[order_cg]] + rank          # slot within core
    src_s = src[order_cg]
    dloc_s = dloc[order_cg]
    ck_s = ck[order_cg]
    core_s = core[order_cg]

    idx16_cores, dstl_cores = [], []
    for cr in range(N_CORES):
        m = core_s == cr
        sl = slot[m]
        # ALL padding uses valid idx 0 (its one-hot row is zero via
        # dstl=-1). Trailing -1s would be trimmed from the ucode's static
        # descriptor count, but ring booking + the HW trigger use the
        # register count - a mismatch makes the DMA consume stale ring
        # descriptors (device fault). Keep them equal: no -1s at all.
        stream = np.zeros(tot_slots, np.int16)
        stream[sl] = (src_s[m] - ck_s[m] * CHUNK).astype(np.int16)
        wrapped = stream.reshape(tot_slots // 16, 16).T   # [16, S/16]
        idx16 = np.tile(wrapped, (8, 1)).astype(np.int16)  # [128, S/16]
        idx16_cores.append(np.ascontiguousarray(idx16))

        dstl = np.full(tot_slots, -1.0, np.float32)
        dstl[sl] = (dloc_s[m] % SW).astype(np.float32)
        dstl = np.ascontiguousarray(dstl.reshape(tot_tiles, P).T)
        dstl_cores.append(dstl)

    # schedule: per call, list of (bl, sw, T, gt0, lt0); per (blk,b) tiles
    call_plan = []
    for k in range(N_CALLS):
        plan = []
        lt0 = 0
        for gi in range(k * GROUPS_PER_CALL, (k + 1) * GROUPS_PER_CALL):
            T = int(T_g[gi])
            if T == 0:
                continue
            rel = gi - k * GROUPS_PER_CALL
            bl_i, sw_i = rel // N_SW, rel % N_SW
            plan.append((bl_i, sw_i, T, int(grp_tile_off[gi]), lt0))
            lt0 += T
        call_plan.append(plan)

    bw_tiles = T_g.reshape(N_BW, N_CHUNKS, N_SW).sum(axis=(1, 2))  # [N_BW]

    sched = dict(
        tot_slots=tot_slots,
        tot_tiles=tot_tiles,
        call_slot0=call_slot0,
        call_slots=call_slots,
        call_plan=call_plan,
        bw_tiles=bw_tiles,
        t_call_max=int(call_slots.max()) // P,
    )
    return sched, idx16_cores, dstl_cores


def _dma_gather_raw(nc, out_ap, in_ap, idxs_ap, num_idxs, elem_size,
                    elem_step, queue_num, reg):
    """InstDMAGatherAnt builder: like nc.gpsimd.dma_gather but without the
    transpose-mode 256B elem granularity assert (the non-transpose ucode
    path generates one descriptor of elem_size bytes per index; only the
    SOURCE ROW STRIDE must be a 256B multiple)."""
    import concourse.mybir as mybir

    eng = nc.gpsimd
    dsz = mybir.dt.size(in_ap.dtype)
    stride_bytes = elem_step * dsz
    assert stride_bytes % 256 == 0 and stride_bytes // 256 < 256
    assert in_ap.ap[0][0] == elem_step
    assert in_ap.ap[-1][1] == out_ap.ap[-1][1] == elem_size
    assert idxs_ap.dtype == mybir.dt.int16
    assert num_idxs % 128 == 0
    _in_ap = eng.lower_ap_dma(in_ap, for_custom_bir_dma=True)
    _idxs_ap = eng.lower_ap(idxs_ap)
    _out_ap = eng.lower_ap(out_ap)
    inst = eng.add_instruction(
        mybir.InstDMAGatherAnt(
            name=nc.get_next_instruction_name(),
            ins=[*_in_ap, _idxs_ap, eng.lower_val_access(reg)],
            outs=[_out_ap],
            transpose=False,
            num_idxs=num_idxs,
            elem_size=elem_size,
            stride_bytes_256=stride_bytes // 256,
            gen_mode=0,
            single_packet=False,
            queue_num=queue_num,
        )
    )
    return inst


def _build_program(sched, repeat=1):
    import concourse.bass as bass
    import concourse.bacc as bacc
    import concourse.mybir as mybir
    import concourse.tile as tile

    tot_slots = sched["tot_slots"]
    tot_tiles = sched["tot_tiles"]
    call_slot0 = sched["call_slot0"]
    call_slots = sched["call_slots"]
    call_plan = sched["call_plan"]
    bw_tiles = sched["bw_tiles"]
    t_call_max = sched["t_call_max"]

    import os
    dbg = int(os.environ.get("KDBG", "0"))
    nc = bacc.Bacc(None, target_bir_lowering=False, debug=False,
                   num_swdge_queues=4, dynamic_dma_scratch_size=32768)
    x_in = nc.declare_dram_parameter("x", [N_NODES, D_FEAT],
                                     mybir.dt.float32, isOutput=False)
    idx_in = nc.declare_dram_parameter("idx", [128, tot_slots // 16],
                                       mybir.dt.int16, isOutput=False)
    dstl_in = nc.declare_dram_parameter("dstl", [128, tot_tiles],
                                        mybir.dt.float32, isOutput=False)
    iota_in = nc.declare_dram_parameter("iota", [128, SW],
                                        mybir.dt.float16, isOutput=False)
    yT_out = nc.declare_dram_parameter("yT", [D_FEAT, R], mybir.dt.float32,
                                       isOutput=True)

    with tile.TileContext(nc) as tc:
        with (
            tc.tile_pool(name="const", bufs=1) as constp,
            tc.tile_pool(name="idxp", bufs=1) as idxp,
            tc.tile_pool(name="dstlp", bufs=1) as dstlp,
            tc.tile_pool(name="outp", bufs=1) as outp,
            tc.tile_pool(name="castp", bufs=4) as castp,
            tc.tile_pool(name="msgp", bufs=1) as msgp,
            tc.tile_pool(name="ohp", bufs=12) as ohp,
            tc.tile_pool(name="x2p", bufs=1, space="DRAM") as x2p,
            tc.tile_pool(name="psp", bufs=8, space="PSUM") as psp,
        ):
            iota_sb = constp.tile([128, SW], mybir.dt.float16)
            nc.sync.dma_start(out=iota_sb[:], in_=iota_in[:, :])
            zero64 = constp.tile([128, D_FEAT], mybir.dt.float16)
            nc.vector.memset(zero64[:], 0.0)
            zrhs = constp.tile([128, BW], mybir.dt.float16)
            nc.vector.memset(zrhs[:], 0.0)
            idx_sb = idxp.tile([128, tot_slots // 16], mybir.dt.int16)
            nc.sync.dma_start(out=idx_sb[:], in_=idx_in[:, :])
            dstl_sb = dstlp.tile([128, tot_tiles], mybir.dt.float32)
            nc.sync.dma_start(out=dstl_sb[:], in_=dstl_in[:, :])
            outT_sb = outp.tile([D_FEAT, R], mybir.dt.float32)

            # one-time: cast x fp32 -> x2 fp16 [N, 128] (payload cols 0:64)
            x2 = x2p.tile([N_NODES, 128], mybir.dt.float16)
            CAST_ROWS = 1024
            CAST_FREE = CAST_ROWS * D_FEAT // 128
            r0 = 0
            while r0 < N_NODES:
                left = N_NODES - r0
                step = CAST_ROWS if left >= CAST_ROWS else (
                    128 if left >= 128 else left
                )
                r1 = r0 + step
                free = (r1 - r0) * D_FEAT // 128
                cin = castp.tile([128, CAST_FREE], mybir.dt.float32,
                                 tag="cin")
                nc.sync.dma_start(out=cin[:, :free], in_=x_in[r0:r1, :])
                c16 = castp.tile([128, CAST_FREE], mybir.dt.float16,
                                 tag="c16")
                nc.vector.tensor_copy(out=c16[:, :free], in_=cin[:, :free])
                nc.sync.dma_start(out=x2[r0:r1, 0:D_FEAT],
                                  in_=c16[:, :free])
                r0 = r1

            # manual msg ring: memset once so trailing-trimmed slots stay
            # finite (garbage x rows are multiplied by a zero one-hot row)
            N_MSG = 3
            msg_bufs = []
            for i in range(N_MSG):
                mb = msgp.tile([128, t_call_max, D_FEAT], mybir.dt.float16,
                               tag=f"msg{i}")
                nc.vector.memset(mb[:], 0.0)
                msg_bufs.append(mb)

            reg_cache = {}

            def reg_for(val):
                if val not in reg_cache:
                    reg_cache[val] = nc.gpsimd.to_reg(val)
                return reg_cache[val]

            qn = 0
            for _rep in range(repeat):
                for blk in range(N_BLK):
                    psums = []
                    for bl_i in range(BLK):
                        ps = psp.tile([128, BW], mybir.dt.float32,
                                      space="PSUM", tag="ps")
                        psT = ps[0:D_FEAT, :]
                        nc.tensor.matmul(out=psT, lhsT=zero64[:],
                                         rhs=zrhs[:], start=True,
                                         stop=(dbg >= 1))
                        psums.append(ps)
                    done = [0] * BLK
                    for c in range(N_CHUNKS):
                        k = blk * N_CHUNKS + c
                        S = int(call_slots[k])
                        if S == 0:
                            continue
                        s0 = int(call_slot0[k])
                        T_call = S // P
                        msg = msg_bufs[qn % N_MSG]
                        if dbg >= 200:
                            emit_gather = qn < dbg - 200
                        elif dbg >= 100:
                            emit_gather = qn == dbg - 100
                        else:
                            emit_gather = dbg < 3
                        if emit_gather:
                            _dma_gather_raw(
                                nc,
                                out_ap=msg[:, :T_call, :],
                                in_ap=x2[c * CHUNK:
                                         min((c + 1) * CHUNK, N_NODES),
                                         0:D_FEAT],
                                idxs_ap=idx_sb[:, s0 // 16:(s0 + S) // 16],
                                num_idxs=S,
                                elem_size=D_FEAT,
                                elem_step=128,
                                queue_num=qn % 4,
                                reg=reg_for(S),
                            )
                        qn += 1
                        for (bl_i, sw_i, T, gt0, lt0) in call_plan[k]:
                            b = blk * BLK + bl_i
                            psT = psums[bl_i][0:D_FEAT, :]
                            for tl in range(T):
                                done[bl_i] += 1
                                if dbg >= 2:
                                    continue
                                oh = ohp.tile([128, SW], mybir.dt.float16,
                                              tag="oh")
                                nc.vector.tensor_scalar(
                                    out=oh[:],
                                    in0=iota_sb[:],
                                    scalar1=dstl_sb[:, gt0 + tl:gt0 + tl + 1],
                                    scalar2=None,
                                    op0=mybir.AluOpType.is_equal,
                                )
                                if dbg >= 1:
                                    continue
                                nc.tensor.matmul(
                                    out=psT[:, sw_i * SW:(sw_i + 1) * SW],
                                    lhsT=msg[:, lt0 + tl, :],
                                    rhs=oh[:],
                                    start=False,
                                    stop=(done[bl_i] == int(bw_tiles[b])),
                                )
                    for bl_i in range(BLK):
                        b = blk * BLK + bl_i
                        nc.scalar.copy(
                            out=outT_sb[:, b * BW:(b + 1) * BW],
                            in_=psums[bl_i][0:D_FEAT, :],
                        )
            nc.sync.dma_start(out=yT_out[:, :], in_=outT_sb[:])
    nc.compile()
    return nc


def build(x, edge_index, repeat=1):
    """Preprocess + build the compiled Bass program and per-core inputs."""
    x = np.ascontiguousarray(np.asarray(x, dtype=np.float32))
    edge_index = np.asarray(edge_index)
    assert x.shape == (N_NODES, D_FEAT), x.shape

    sched, idx16_cores, dstl_cores = _preprocess(edge_index)
    nc = _build_program(sched, repeat=repeat)

    iota = np.tile(np.arange(SW, dtype=np.float16), (128, 1))
    in_maps = []
    for cr in range(N_CORES):
        in_maps.append({
            "x": x,
            "idx": idx16_cores[cr],
            "dstl": dstl_cores[cr],
            "iota": iota,
        })
    return nc, in_maps


def postprocess(results, n_nodes):
    out = np.empty((N_CORES * R, D_FEAT), np.float32)
    for cr in range(N_CORES):
        out[cr * R:(cr + 1) * R] = results[cr]["yT"].T
    return out[:n_nodes]


def kernel(x, edge_index):
    n_nodes = np.asarray(x).shape[0]
    nc, in_maps = build(x, edge_index)
    from concourse.bass_utils import run_bass_kernel_spmd
    res = run_bass_kernel_spmd(nc, in_maps, list(range(N_CORES)))
    return postprocess(res.results, n_nodes)


if __name__ == "__main__":
    import reference
    inputs = reference.setup_inputs()
    inputs = {k: np.asarray(v) for k, v in inputs.items()}
    got = kernel(**inputs)
    want = np.asarray(reference.reference(**{k: v for k, v in inputs.items()}))
    denom = max(np.abs(want).max(), 1e-30)
    rel = np.abs(got - want).max() / denom
    print(f"Relative error: {rel:.3e}")


# revision 16
# speedup vs baseline: 1.4868x; 1.4868x over previous
"""GNN message-passing kernel for Trainium2 (8 NeuronCores, SPMD).

Computes: out = segment_sum(x[edge_index[0]], edge_index[1], num_segments=N)
  i.e. for each edge e: out[dst[e]] += x[src[e]]

Strategy (v2):
  - Shard destination nodes across 8 cores (R=12800 dst nodes/core).
  - One-time on device (outside the repeat loop): cast x fp32 -> x2 fp16
    stored [N, 128] (256B row stride, payload in cols 0..63).
  - Per edge: gather its 128B fp16 source row from x2 via the Q7 dma_gather
    (raw InstDMAGatherAnt: elem_size=64 fp16, elem_step=128 -> 128B
    descriptors; the non-transpose ucode path has no 256B granularity
    requirement).
  - Edges sorted by (bigwin-block, src chunk, bigwin, subwindow, dst).
    Chunks are 25000 src rows (int16 gather index limit). ONE gather call
    per (block of 5 bigwins, chunk) = 20 calls/core, so the ~1us fixed
    SWDGE generation cost per call amortizes. Padding slots use a VALID
    index 0 with dstl=-1 (one-hot row all zero -> contributes nothing),
    letting calls span group boundaries; only each call's trailing pad
    run uses -1 (trimmed for free by the ucode).
  - Scatter: per 128-edge tile, DVE tensor_scalar is_equal(iota_fp16,
    dstl_scalar) builds a [128,128] fp16 one-hot (TensorScalarPtr hits the
    4x_2p DVE mode); PE fp16 matmul accumulates msgs^T @ onehot into a
    feature-major PSUM bank [64 feat, 512 dst] per bigwin (5 banks live
    per block).
  - Flush PSUM -> SBUF outT [64, 12800] fp32 on the Activation engine ->
    single DMA out; host transposes.

The Bass program is identical across cores (SPMD): tile counts use the max
over cores; per-core variation lives in the idx / dstl input streams.
"""
import numpy as np

N_NODES = 100000
D_FEAT = 64
N_CORES = 8
R = 12800            # dst nodes per core (8*R = 102400 >= N_NODES)
BW = 512             # bigwin: dst nodes per PSUM bank
SW = 128             # subwindow: one-hot window width
CHUNK = 25000        # src rows per gather base (int16 idx limit 32767)
N_CHUNKS = 4
N_BW = R // BW       # 25 bigwins per core
N_SW = BW // SW      # 4 subwindows per bigwin
P = 128              # edge slots per tile
BLK = 5              # bigwins per block (PSUM banks live at once)
N_BLK = N_BW // BLK  # 5 blocks
GROUPS_PER_CALL = BLK * N_SW          # 20 (b, sw) groups per (blk, c) call
N_CALLS = N_BLK * N_CHUNKS            # 20 calls
N_GROUPS = N_BW * N_CHUNKS * N_SW     # 400


def cdiv(a, b):
    return -(-a // b)


def _preprocess(edge_index):
    """Sort/partition edges; build per-core device input streams and the
    (core-uniform) schedule. Purely graph-structural (no x values)."""
    src = np.asarray(edge_index[0], dtype=np.int64)
    dst = np.asarray(edge_index[1], dtype=np.int64)
    E = src.shape[0]

    core = dst // R
    dloc = dst % R
    b = dloc // BW                    # bigwin within core
    sw = (dloc % BW) // SW            # subwindow within bigwin
    ck = src // CHUNK                 # src chunk
    blk = b // BLK
    bl = b % BLK                      # bigwin within block

    # group key ordered call-major: call = (blk, ck), inside call (bl, sw)
    call = blk * N_CHUNKS + ck
    gkey = (call * BLK + bl) * N_SW + sw
    # per (core, gkey) counts
    cg = core * np.int64(N_GROUPS) + gkey
    counts = np.bincount(cg, minlength=N_CORES * N_GROUPS).reshape(
        N_CORES, N_GROUPS
    )
    max_counts = counts.max(axis=0)                     # [N_GROUPS]
    T_g = cdiv(max_counts, P)                           # tiles per group
    grp_tile_off = np.zeros(N_GROUPS, np.int64)
    np.cumsum(T_g[:-1], out=grp_tile_off[1:])
    grp_slot_off = grp_tile_off * P
    tot_tiles = int(T_g.sum())
    tot_slots = tot_tiles * P

    # call boundaries (slots): call k covers gkeys [k*GPC, (k+1)*GPC)
    call_g0 = np.arange(N_CALLS) * GROUPS_PER_CALL
    call_slot0 = grp_slot_off[call_g0]
    call_slots = np.array(
        [
            int(
                T_g[k * GROUPS_PER_CALL:(k + 1) * GROUPS_PER_CALL].sum() * P
            )
            for k in range(N_CALLS)
        ],
        np.int64,
    )

    # per-edge slot assignment: rank within each (core, gkey) group
    order_cg = np.argsort(cg, kind="stable")
    cg_s = cg[order_cg]
    grp_start = np.zeros(N_CORES * N_GROUPS + 1, np.int64)
    np.cumsum(counts.reshape(-1), out=grp_start[1:])
    rank = np.arange(E, dtype=np.int64) - grp_start[cg_s]
    slot = grp_slot_off[gkey[order_cg]] + rank          # slot within core
    src_s = src[order_cg]
    dloc_s = dloc[order_cg]
    ck_s = ck[order_cg]
    core_s = core[order_cg]

    idx16_cores, dstl_cores = [], []
    for cr in range(N_CORES):
        m = core_s == cr
        sl = slot[m]
        # ALL padding uses VALID indices (one-hot row is zero via
        # dstl=-1). Trailing -1s would be trimmed from the ucode's static
        # descriptor count, but ring booking + the HW trigger use the
        # register count - a mismatch makes the DMA consume stale ring
        # descriptors (device fault). Keep them equal: no -1s at all.
        # Spread pad reads across rows: same-row pad reads hotspot one
        # HBM row and head-of-line-block the in-order DMA engines
        # (measured 2.7x gather slowdown with idx=0 padding).
        stream = ((np.arange(tot_slots, dtype=np.int64) * 9973) %
                  CHUNK).astype(np.int16)
        stream[sl] = (src_s[m] - ck_s[m] * CHUNK).astype(np.int16)
        wrapped = stream.reshape(tot_slots // 16, 16).T   # [16, S/16]
        idx16 = np.tile(wrapped, (8, 1)).astype(np.int16)  # [128, S/16]
        idx16_cores.append(np.ascontiguousarray(idx16))

        dstl = np.full(tot_slots, -1.0, np.float32)
        dstl[sl] = (dloc_s[m] % SW).astype(np.float32)
        dstl = np.ascontiguousarray(dstl.reshape(tot_tiles, P).T)
        dstl_cores.append(dstl)

    # schedule: per call, list of (bl, sw, T, gt0, lt0); per (blk,b) tiles
    call_plan = []
    for k in range(N_CALLS):
        plan = []
        lt0 = 0
        for gi in range(k * GROUPS_PER_CALL, (k + 1) * GROUPS_PER_CALL):
            T = int(T_g[gi])
            if T == 0:
                continue
            rel = gi - k * GROUPS_PER_CALL
            bl_i, sw_i = rel // N_SW, rel % N_SW
            plan.append((bl_i, sw_i, T, int(grp_tile_off[gi]), lt0))
            lt0 += T
        call_plan.append(plan)

    bw_tiles = T_g.reshape(N_BW, N_CHUNKS, N_SW).sum(axis=(1, 2))  # [N_BW]

    sched = dict(
        tot_slots=tot_slots,
        tot_tiles=tot_tiles,
        call_slot0=call_slot0,
        call_slots=call_slots,
        call_plan=call_plan,
        bw_tiles=bw_tiles,
        t_call_max=int(call_slots.max()) // P,
    )
    return sched, idx16_cores, dstl_cores


def _dma_gather_raw(nc, out_ap, in_ap, idxs_ap, num_idxs, elem_size,
                    elem_step, queue_num, reg):
    """InstDMAGatherAnt builder: like nc.gpsimd.dma_gather but without the
    transpose-mode 256B elem granularity assert (the non-transpose ucode
    path generates one descriptor of elem_size bytes per index; only the
    SOURCE ROW STRIDE must be a 256B multiple)."""
    import concourse.mybir as mybir

    eng = nc.gpsimd
    dsz = mybir.dt.size(in_ap.dtype)
    stride_bytes = elem_step * dsz
    assert stride_bytes % 256 == 0 and stride_bytes // 256 < 256
    assert in_ap.ap[0][0] == elem_step
    assert in_ap.ap[-1][1] == out_ap.ap[-1][1] == elem_size
    assert idxs_ap.dtype == mybir.dt.int16
    assert num_idxs % 128 == 0
    _in_ap = eng.lower_ap_dma(in_ap, for_custom_bir_dma=True)
    _idxs_ap = eng.lower_ap(idxs_ap)
    _out_ap = eng.lower_ap(out_ap)
    inst = eng.add_instruction(
        mybir.InstDMAGatherAnt(
            name=nc.get_next_instruction_name(),
            ins=[*_in_ap, _idxs_ap, eng.lower_val_access(reg)],
            outs=[_out_ap],
            transpose=False,
            num_idxs=num_idxs,
            elem_size=elem_size,
            stride_bytes_256=stride_bytes // 256,
            gen_mode=0,
            single_packet=False,
            queue_num=queue_num,
        )
    )
    return inst


def _build_program(sched, repeat=1):
    import concourse.bass as bass
    import concourse.bacc as bacc
    import concourse.mybir as mybir
    import concourse.tile as tile

    tot_slots = sched["tot_slots"]
    tot_tiles = sched["tot_tiles"]
    call_slot0 = sched["call_slot0"]
    call_slots = sched["call_slots"]
    call_plan = sched["call_plan"]
    bw_tiles = sched["bw_tiles"]
    t_call_max = sched["t_call_max"]

    import os
    dbg = int(os.environ.get("KDBG", "0"))
    nc = bacc.Bacc(None, target_bir_lowering=False, debug=False,
                   num_swdge_queues=4, dynamic_dma_scratch_size=32768)
    x_in = nc.declare_dram_parameter("x", [N_NODES, D_FEAT],
                                     mybir.dt.float32, isOutput=False)
    idx_in = nc.declare_dram_parameter("idx", [128, tot_slots // 16],
                                       mybir.dt.int16, isOutput=False)
    dstl_in = nc.declare_dram_parameter("dstl", [128, tot_tiles],
                                        mybir.dt.float32, isOutput=False)
    iota_in = nc.declare_dram_parameter("iota", [128, SW],
                                        mybir.dt.float16, isOutput=False)
    yT_out = nc.declare_dram_parameter("yT", [D_FEAT, R], mybir.dt.float32,
                                       isOutput=True)

    with tile.TileContext(nc) as tc:
        with (
            tc.tile_pool(name="const", bufs=1) as constp,
            tc.tile_pool(name="idxp", bufs=1) as idxp,
            tc.tile_pool(name="dstlp", bufs=1) as dstlp,
            tc.tile_pool(name="outp", bufs=1) as outp,
            tc.tile_pool(name="castp", bufs=4) as castp,
            tc.tile_pool(name="msgp", bufs=1) as msgp,
            tc.tile_pool(name="ohp", bufs=12) as ohp,
            tc.tile_pool(name="x2p", bufs=1, space="DRAM") as x2p,
            tc.tile_pool(name="psp", bufs=8, space="PSUM") as psp,
        ):
            iota_sb = constp.tile([128, SW], mybir.dt.float16)
            nc.sync.dma_start(out=iota_sb[:], in_=iota_in[:, :])
            zero64 = constp.tile([128, D_FEAT], mybir.dt.float16)
            nc.vector.memset(zero64[:], 0.0)
            zrhs = constp.tile([128, BW], mybir.dt.float16)
            nc.vector.memset(zrhs[:], 0.0)
            idx_sb = idxp.tile([128, tot_slots // 16], mybir.dt.int16)
            nc.sync.dma_start(out=idx_sb[:], in_=idx_in[:, :])
            dstl_sb = dstlp.tile([128, tot_tiles], mybir.dt.float32)
            nc.sync.dma_start(out=dstl_sb[:], in_=dstl_in[:, :])
            outT_sb = outp.tile([D_FEAT, R], mybir.dt.float32)

            # one-time: cast x fp32 -> x2 fp16 [N, 128] (payload cols 0:64)
            x2 = x2p.tile([N_NODES, 128], mybir.dt.float16)
            CAST_ROWS = 1024
            CAST_FREE = CAST_ROWS * D_FEAT // 128
            r0 = 0
            while r0 < N_NODES:
                left = N_NODES - r0
                step = CAST_ROWS if left >= CAST_ROWS else (
                    128 if left >= 128 else left
                )
                r1 = r0 + step
                free = (r1 - r0) * D_FEAT // 128
                cin = castp.tile([128, CAST_FREE], mybir.dt.float32,
                                 tag="cin")
                nc.sync.dma_start(out=cin[:, :free], in_=x_in[r0:r1, :])
                c16 = castp.tile([128, CAST_FREE], mybir.dt.float16,
                                 tag="c16")
                nc.vector.tensor_copy(out=c16[:, :free], in_=cin[:, :free])
                nc.sync.dma_start(out=x2[r0:r1, 0:D_FEAT],
                                  in_=c16[:, :free])
                r0 = r1

            # manual msg ring: memset once so trailing-trimmed slots stay
            # finite (garbage x rows are multiplied by a zero one-hot row)
            N_MSG = 3
            msg_bufs = []
            for i in range(N_MSG):
                mb = msgp.tile([128, t_call_max, D_FEAT], mybir.dt.float16,
                               tag=f"msg{i}")
                nc.vector.memset(mb[:], 0.0)
                msg_bufs.append(mb)

            reg_cache = {}

            def reg_for(val):
                if val not in reg_cache:
                    reg_cache[val] = nc.gpsimd.to_reg(val)
                return reg_cache[val]

            qn = 0
            for _rep in range(repeat):
                for blk in range(N_BLK):
                    psums = []
                    for bl_i in range(BLK):
                        ps = psp.tile([128, BW], mybir.dt.float32,
                                      space="PSUM", tag="ps")
                        psT = ps[0:D_FEAT, :]
                        nc.tensor.matmul(out=psT, lhsT=zero64[:],
                                         rhs=zrhs[:], start=True,
                                         stop=(dbg >= 1))
                        psums.append(ps)
                    done = [0] * BLK
                    for c in range(N_CHUNKS):
                        k = blk * N_CHUNKS + c
                        S = int(call_slots[k])
                        if S == 0:
                            continue
                        s0 = int(call_slot0[k])
                        T_call = S // P
                        msg = msg_bufs[qn % N_MSG]
                        if dbg >= 200:
                            emit_gather = qn < dbg - 200
                        elif dbg >= 100:
                            emit_gather = qn == dbg - 100
                        else:
                            emit_gather = dbg < 3
                        if emit_gather:
                            _dma_gather_raw(
                                nc,
                                out_ap=msg[:, :T_call, :],
                                in_ap=x2[c * CHUNK:
                                         min((c + 1) * CHUNK, N_NODES),
                                         0:D_FEAT],
                                idxs_ap=idx_sb[:, s0 // 16:(s0 + S) // 16],
                                num_idxs=S,
                                elem_size=D_FEAT,
                                elem_step=128,
                                queue_num=qn % 4,
                                reg=reg_for(S),
                            )
                        qn += 1
                        for (bl_i, sw_i, T, gt0, lt0) in call_plan[k]:
                            b = blk * BLK + bl_i
                            psT = psums[bl_i][0:D_FEAT, :]
                            for tl in range(T):
                                done[bl_i] += 1
                                if dbg >= 2:
                                    continue
                                oh = ohp.tile([128, SW], mybir.dt.float16,
                                              tag="oh")
                                nc.vector.tensor_scalar(
                                    out=oh[:],
                                    in0=iota_sb[:],
                                    scalar1=dstl_sb[:, gt0 + tl:gt0 + tl + 1],
                                    scalar2=None,
                                    op0=mybir.AluOpType.is_equal,
                                )
                                if dbg >= 1:
                                    continue
                                nc.tensor.matmul(
                                    out=psT[:, sw_i * SW:(sw_i + 1) * SW],
                                    lhsT=msg[:, lt0 + tl, :],
                                    rhs=oh[:],
                                    start=False,
                                    stop=(done[bl_i] == int(bw_tiles[b])),
                                )
                    for bl_i in range(BLK):
                        b = blk * BLK + bl_i
                        nc.scalar.copy(
                            out=outT_sb[:, b * BW:(b + 1) * BW],
                            in_=psums[bl_i][0:D_FEAT, :],
                        )
            nc.sync.dma_start(out=yT_out[:, :], in_=outT_sb[:])
    nc.compile()
    return nc


def build(x, edge_index, repeat=1):
    """Preprocess + build the compiled Bass program and per-core inputs."""
    x = np.ascontiguousarray(np.asarray(x, dtype=np.float32))
    edge_index = np.asarray(edge_index)
    assert x.shape == (N_NODES, D_FEAT), x.shape

    sched, idx16_cores, dstl_cores = _preprocess(edge_index)
    nc = _build_program(sched, repeat=repeat)

    iota = np.tile(np.arange(SW, dtype=np.float16), (128, 1))
    in_maps = []
    for cr in range(N_CORES):
        in_maps.append({
            "x": x,
            "idx": idx16_cores[cr],
            "dstl": dstl_cores[cr],
            "iota": iota,
        })
    return nc, in_maps


def postprocess(results, n_nodes):
    out = np.empty((N_CORES * R, D_FEAT), np.float32)
    for cr in range(N_CORES):
        out[cr * R:(cr + 1) * R] = results[cr]["yT"].T
    return out[:n_nodes]


def kernel(x, edge_index):
    n_nodes = np.asarray(x).shape[0]
    nc, in_maps = build(x, edge_index)
    from concourse.bass_utils import run_bass_kernel_spmd
    res = run_bass_kernel_spmd(nc, in_maps, list(range(N_CORES)))
    return postprocess(res.results, n_nodes)


if __name__ == "__main__":
    import reference
    inputs = reference.setup_inputs()
    inputs = {k: np.asarray(v) for k, v in inputs.items()}
    got = kernel(**inputs)
    want = np.asarray(reference.reference(**{k: v for k, v in inputs.items()}))
    denom = max(np.abs(want).max(), 1e-30)
    rel = np.abs(got - want).max() / denom
    print(f"Relative error: {rel:.3e}")


# revision 17
# speedup vs baseline: 3.7475x; 2.5206x over previous
"""GNN message-passing kernel for Trainium2 (8 NeuronCores, SPMD).

Computes: out = segment_sum(x[edge_index[0]], edge_index[1], num_segments=N)
  i.e. for each edge e: out[dst[e]] += x[src[e]]

Strategy:
  - Shard destination nodes across 8 cores (R=12800 nodes/core, padded space).
  - x replicated; each core gathers its edges' source rows from HBM via the
    custom Q7 dma_gather (int16 idx, 4 chunk bases of 32768 rows, 4 SWDGE
    queues for parallel descriptor service).
  - Edges sorted by (dst bigwin, src chunk, dst). One gather call per
    (bigwin=512 dst nodes, chunk). Tiles of 128 edge slots aligned to
    subwindows (128 dst nodes) with -1 slot padding (free: skipped by DMA).
  - Scatter: per tile, DVE builds a one-hot [128 edges, 128 nodes] via
    is_equal(dst_local, iota); PE matmul accumulates msgs^T @ onehot into a
    feature-major PSUM bank [64 feat, 512 nodes] per bigwin.
  - Flush PSUM -> SBUF outT [64, 12800] -> single DMA out; host transposes.

The Bass program is identical across cores (SPMD): tile counts use the max
over cores; per-core valid-index counts are runtime inputs consumed via
reg_load into the dma_gather num_idxs_reg.
"""
import numpy as np

N_NODES = 100000
D_FEAT = 64
N_CORES = 8
R = 12800            # dst nodes per core (8*R = 102400 >= N_NODES)
BW = 512             # bigwin: dst nodes per PSUM bank
SW = 128             # subwindow: dst nodes per tile target / onehot width
CHUNK = 32768        # src rows per gather base (int16 idx limit)
N_CHUNKS = 4         # ceil(100000 / 32768)
N_BW = R // BW       # 25 bigwins per core
N_SW = BW // SW      # 4 subwindows per bigwin
P = 128              # edge slots per tile


def cdiv(a, b):
    return -(-a // b)


def _preprocess(x, edge_index):
    """Sort/partition edges; build per-core device input streams and the
    (core-uniform) schedule."""
    src = np.asarray(edge_index[0], dtype=np.int64)
    dst = np.asarray(edge_index[1], dtype=np.int64)
    E = src.shape[0]

    core = dst // R
    dloc = dst % R                 # dst local to core
    bw = dloc // BW                # bigwin within core
    sw = (dloc % BW) // SW         # subwindow within bigwin
    ck = src // CHUNK              # src chunk

    # group id per edge: (core, bw, ck, sw)
    gid = ((core * N_BW + bw) * N_CHUNKS + ck) * N_SW + sw
    n_groups = N_CORES * N_BW * N_CHUNKS * N_SW
    order = np.argsort(gid, kind="stable")
    gid_s = gid[order]
    src_s = src[order]
    dloc_s = dloc[order]

    counts = np.bincount(gid_s, minlength=n_groups)           # [n_groups]
    counts4 = counts.reshape(N_CORES, N_BW, N_CHUNKS, N_SW)
    # tiles per (bw, ck, sw): max over cores (uniform program)
    T_g = cdiv(counts4.max(axis=0), P)                        # [N_BW, N_CHUNKS, N_SW]

    # schedule: one gather call per nonempty group (bw, ck, sw);
    # padding is a -1 suffix per call (trimmed by the Q7 ucode, no DMA cost).
    tiles_per_call = T_g.sum(axis=2)                          # [N_BW, N_CHUNKS]
    n_calls = N_BW * N_CHUNKS
    tot_tiles = int(T_g.sum())
    tot_slots = tot_tiles * P

    # slot offset of each group (bw, ck, sw) within the global stream
    grp_tile_off = np.zeros((N_BW, N_CHUNKS, N_SW), np.int64)
    acc = 0
    for b in range(N_BW):
        for c in range(N_CHUNKS):
            for s in range(N_SW):
                grp_tile_off[b, c, s] = acc
                acc += T_g[b, c, s]
    assert acc == tot_tiles
    grp_slot_off = grp_tile_off * P

    # tile metadata (uniform across cores): subwindow index per global tile
    tile_sw = np.zeros(tot_tiles, np.int64)
    tile_bw = np.zeros(tot_tiles, np.int64)
    tile_call = np.zeros(tot_tiles, np.int64)
    for b in range(N_BW):
        for c in range(N_CHUNKS):
            for s in range(N_SW):
                o = grp_tile_off[b, c, s]
                t = T_g[b, c, s]
                tile_sw[o:o + t] = s
                tile_bw[o:o + t] = b
                tile_call[o:o + t] = b * N_CHUNKS + c

    # per-edge slot assignment (vectorized)
    # rank of edge within its group:
    grp_start_edge = np.zeros(n_groups + 1, np.int64)
    np.cumsum(counts, out=grp_start_edge[1:])
    rank = np.arange(E, dtype=np.int64) - grp_start_edge[gid_s]
    b_e = (gid_s // (N_CHUNKS * N_SW)) % N_BW
    c_e = (gid_s // N_SW) % N_CHUNKS
    s_e = gid_s % N_SW
    slot = grp_slot_off[b_e, c_e, s_e] + rank                  # within-core slot
    core_e = gid_s // (N_BW * N_CHUNKS * N_SW)

    # build per-core streams
    idx16_cores, dstl_cores, cnt_cores = [], [], []
    for cr in range(N_CORES):
        m = core_e == cr
        sl = slot[m]
        stream = np.full(tot_slots, -1, np.int16)
        lidx = (src_s[m] - c_e[m] * CHUNK).astype(np.int16)
        stream[sl] = lidx
        # wrapped-16 idx layout, replicated across 8 groups of 16 partitions
        wrapped = stream.reshape(tot_slots // 16, 16).T        # [16, S/16]
        idx16 = np.tile(wrapped, (8, 1)).astype(np.int16)      # [128, S/16]
        idx16_cores.append(idx16)

        dstl = np.full(tot_slots, -1.0, np.float32)
        dstl[sl] = (dloc_s[m] % SW).astype(np.float32)
        dstl = dstl.reshape(tot_tiles, P).T.copy()             # [128, tot_tiles]
        dstl_cores.append(dstl)

        # valid count per group, flattened (b, c, s)
        cnt = counts4[cr].reshape(-1).astype(np.int32)
        cnt_cores.append(cnt.reshape(1, -1))

    sched = dict(
        T_g=T_g, tiles_per_call=tiles_per_call,
        grp_tile_off=grp_tile_off,
        tot_slots=tot_slots, tot_tiles=tot_tiles, n_calls=n_calls,
        tile_sw=tile_sw, tile_bw=tile_bw, tile_call=tile_call,
    )
    return sched, idx16_cores, dstl_cores, cnt_cores


def _build_program(sched, n_x_rows, repeat=1):
    import concourse.bass as bass
    import concourse.bacc as bacc
    import concourse.mybir as mybir
    import concourse.tile as tile

    tot_slots = sched["tot_slots"]
    tot_tiles = sched["tot_tiles"]
    T_g = sched["T_g"]
    grp_tile_off = sched["grp_tile_off"]

    max_grp_tiles = int(T_g.max())

    nc = bacc.Bacc(None, target_bir_lowering=False, debug=False,
                   num_swdge_queues=4)
    x_in = nc.declare_dram_parameter("x", [n_x_rows, D_FEAT], mybir.dt.float32,
                                     isOutput=False)
    idx_in = nc.declare_dram_parameter("idx", [128, tot_slots // 16],
                                       mybir.dt.int16, isOutput=False)
    dstl_in = nc.declare_dram_parameter("dstl", [128, tot_tiles],
                                        mybir.dt.float32, isOutput=False)
    iota_in = nc.declare_dram_parameter("iota", [128, SW], mybir.dt.float32,
                                        isOutput=False)
    n_groups = N_BW * N_CHUNKS * N_SW
    cnt_in = nc.declare_dram_parameter("cnt", [1, n_groups], mybir.dt.int32,
                                       isOutput=False)
    yT_out = nc.declare_dram_parameter("yT", [D_FEAT, R], mybir.dt.float32,
                                       isOutput=True)

    with tile.TileContext(nc) as tc:
        with (
            tc.tile_pool(name="const", bufs=1) as constp,
            tc.tile_pool(name="idxp", bufs=1) as idxp,
            tc.tile_pool(name="dstlp", bufs=1) as dstlp,
            tc.tile_pool(name="outp", bufs=1) as outp,
            tc.tile_pool(name="msgp", bufs=1) as msgp,
            tc.tile_pool(name="ohp", bufs=8) as ohp,
            tc.tile_pool(name="psp", bufs=4, space="PSUM") as psp,
        ):
            iota_sb = constp.tile([128, SW], mybir.dt.float32)
            nc.sync.dma_start(out=iota_sb[:], in_=iota_in[:, :])
            zero64 = constp.tile([128, D_FEAT], mybir.dt.float32)
            nc.gpsimd.memset(zero64[:], 0.0)
            zrhs = constp.tile([128, BW], mybir.dt.float32)
            nc.gpsimd.memset(zrhs[:], 0.0)
            idx_sb = idxp.tile([128, tot_slots // 16], mybir.dt.int16)
            nc.sync.dma_start(out=idx_sb[:], in_=idx_in[:, :])
            dstl_sb = dstlp.tile([128, tot_tiles], mybir.dt.float32)
            nc.sync.dma_start(out=dstl_sb[:], in_=dstl_in[:, :])
            outT_sb = outp.tile([D_FEAT, R], mybir.dt.float32)
            cnt_sb = constp.tile([1, n_groups], mybir.dt.int32)
            nc.sync.dma_start(out=cnt_sb[:], in_=cnt_in[:, :])
            reg = nc.gpsimd.alloc_register("nval")
            prev_gather = None

            # manual msg ring: memset once so -1-padded slots stay finite
            N_MSG_BUFS = 12
            msg_bufs = []
            for i in range(N_MSG_BUFS):
                mb = msgp.tile([128, max_grp_tiles, D_FEAT], mybir.dt.float32,
                               tag=f"msg{i}")
                nc.vector.memset(mb[:], 0.0)
                msg_bufs.append(mb)

            qn = 0
            for _rep in range(repeat):
              for b in range(N_BW):
                  psumT_full = psp.tile([128, BW], mybir.dt.float32, space="PSUM")
                  psumT = psumT_full[0:D_FEAT, :]
                  # zero-fill the bank (handles zero-edge node columns)
                  nc.tensor.matmul(out=psumT, lhsT=zero64[:], rhs=zrhs[:],
                                   start=True, stop=False)
                  bw_tiles = int(sched["tiles_per_call"][b, :].sum())
                  done = 0
                  for c in range(N_CHUNKS):
                      for sw_i in range(N_SW):
                          T = int(T_g[b, c, sw_i])
                          if T == 0:
                              continue
                          gt0 = int(grp_tile_off[b, c, sw_i])
                          s0 = gt0 * P
                          S = T * P
                          msg = msg_bufs[qn % N_MSG_BUFS]
                          gidx = (b * N_CHUNKS + c) * N_SW + sw_i
                          ld = nc.gpsimd.reg_load(reg, cnt_sb[0:1, gidx:gidx + 1])
                          g = nc.gpsimd.dma_gather(
                              out_ap=msg[:, :T, :],
                              in_ap=x_in[c * CHUNK:, :],
                              idxs_ap=idx_sb[:, s0 // 16:(s0 + S) // 16],
                              num_idxs=S,
                              num_idxs_reg=reg,
                              elem_size=D_FEAT,
                              single_packet=False,
                              queue_num=qn % 4,
                          )
                          tile.add_dep_helper(g.ins, ld.ins, sync=False,
                                              reason="gather reads nval reg")
                          if prev_gather is not None:
                              tile.add_dep_helper(ld.ins, prev_gather.ins,
                                                  sync=False,
                                                  reason="reg reuse ordering")
                          prev_gather = g
                          qn += 1
                          for tl in range(T):
                              gt = gt0 + tl
                              oh = ohp.tile([128, SW], mybir.dt.float32, tag="oh")
                              nc.vector.tensor_tensor(
                                  out=oh[:],
                                  in0=dstl_sb[:, gt:gt + 1].to_broadcast([128, SW]),
                                  in1=iota_sb[:],
                                  op=mybir.AluOpType.is_equal,
                              )
                              done += 1
                              nc.tensor.matmul(
                                  out=psumT[0:D_FEAT, sw_i * SW:(sw_i + 1) * SW],
                                  lhsT=msg[:, tl, :],
                                  rhs=oh[:],
                                  start=False,
                                  stop=(done == bw_tiles),
                              )
                  nc.vector.tensor_copy(out=outT_sb[:, b * BW:(b + 1) * BW],
                                        in_=psumT)
            nc.sync.dma_start(out=yT_out[:, :], in_=outT_sb[:])
    nc.compile()
    return nc


def build(x, edge_index, repeat=1):
    """Preprocess + build the compiled Bass program and per-core input maps."""
    x = np.ascontiguousarray(np.asarray(x, dtype=np.float32))
    edge_index = np.asarray(edge_index)
    assert x.shape[1] == D_FEAT, x.shape

    sched, idx16_cores, dstl_cores, cnt_cores = _preprocess(x, edge_index)
    nc = _build_program(sched, x.shape[0], repeat=repeat)

    iota = np.tile(np.arange(SW, dtype=np.float32), (128, 1))
    in_maps = []
    for cr in range(N_CORES):
        in_maps.append({
            "x": x,
            "idx": idx16_cores[cr],
            "dstl": dstl_cores[cr],
            "cnt": cnt_cores[cr],
            "iota": iota,
        })
    return nc, in_maps


def postprocess(results, n_nodes):
    out = np.empty((N_CORES * R, D_FEAT), np.float32)
    for cr in range(N_CORES):
        out[cr * R:(cr + 1) * R] = results[cr]["yT"].T
    return out[:n_nodes]


def kernel(x, edge_index):
    n_nodes = np.asarray(x).shape[0]
    nc, in_maps = build(x, edge_index)
    from concourse.bass_utils import run_bass_kernel_spmd
    res = run_bass_kernel_spmd(nc, in_maps, list(range(N_CORES)))
    return postprocess(res.results, n_nodes)


if __name__ == "__main__":
    import reference
    inputs = reference.setup_inputs()
    inputs = {k: np.asarray(v) for k, v in inputs.items()}
    got = kernel(**inputs)
    want = np.asarray(reference.reference(**{k: v for k, v in inputs.items()}))
    denom = max(np.abs(want).max(), 1e-30)
    rel = np.abs(got - want).max() / denom
    print(f"Relative error: {rel:.3e}")

